# revision 6
# baseline (speedup 1.0000x reference)
"""ContextQueryAttention Trainium2 kernel.

Reference computation (per batch b):
    S = (c@wc)[:,None] + (q@wq)[None,:] + (c*wm) @ q.T        # (Lc, Lq)
    S1 = softmax(S, axis=0)  (over context dim i)
    S2 = softmax(S, axis=1)  (over question dim j)
    A  = S1 @ q
    Bm = (S1 @ S2.T) @ c
    out = [c, A, c*A, c*Bm] @ wr + br

Algebraic restructuring used here:
  * Bm = S1 @ (S2.T @ c)   -- avoids the (Lc,Lc) intermediate entirely.
  * q~ = wm*q + wc (per-feature). Then q~ @ c.T = M^T + u[i] where
    u = c@wc; the v[j] = q@wq term is constant along the i-softmax and
    cancels, so S1^T = softmax_free(q~ @ c.T) directly.
  * c @ q~.T = M + u[i]; u[i] is constant along the j-softmax, and v[j]
    is added via a rank-1 matmul (ones x v) into the PSUM accumulation.
  * exp() without max subtraction (inputs are unit-scale gaussians; S
    stays |S| < ~10, far from fp32 overflow).
  * softmax normalizers are folded into downstream operands instead of
    rescaling the big exp(S^T) matrix: A = (q/s1) @ E1, Bm uses
    Y = (S2^T c)/s1.
  * The (Lc,4D)@(4D,D) output projection is done blockwise from the
    d-major (transposed) layouts produced naturally by the PE.
  * Matmuls run in float32r (e8m11, full PE rate) -- all PE operands are
    float32r-typed tiles so producers emit rounded values.

Sharding: pure data parallel over batch: 16 batches -> 8 cores x 2.
"""

import numpy as np

import concourse.bass as bass
import concourse.tile as tile
from concourse import bacc, mybir
from concourse import bass2jax
from concourse.masks import make_identity

N_CORES = 8
B, Lc, Lq, D = 16, 2048, 512, 512
BPC = B // N_CORES  # batches per core

F32 = mybir.dt.float32
F32R = mybir.dt.float32r

AF = mybir.ActivationFunctionType
ALU = mybir.AluOpType
AX = mybir.AxisListType

NT = Lc // 128   # 16 context row-blocks
NG = Lq // 128   # 4 question row-blocks
NK = D // 128    # 4 feature blocks
NC_ = Lc // 512  # 4 i-chunks of 512


def build_program(mm_f32r=True):
    MD = F32R if mm_f32r else F32  # dtype of every PE-consumed tile

    nc = bacc.Bacc(None, target_bir_lowering=False)

    c2 = nc.declare_dram_parameter("c2", [BPC, Lc, D], F32, isOutput=False)
    q2 = nc.declare_dram_parameter("q2", [BPC, Lq, D], F32, isOutput=False)
    w0 = nc.declare_dram_parameter("w0", [3 * D], F32, isOutput=False)
    wr = nc.declare_dram_parameter("wr", [4 * D, D], F32, isOutput=False)
    br = nc.declare_dram_parameter("br", [D], F32, isOutput=False)
    out2 = nc.declare_dram_parameter("out2", [BPC, Lc, D], F32, isOutput=True)

    # gpsimd (SWDGE) DMAs can cast f32 -> f32r on the fly; HWDGE cannot.
    def load(out, in_):
        if mm_f32r:
            nc.gpsimd.dma_start(out=out, in_=in_)
        else:
            nc.sync.dma_start(out=out, in_=in_)

    with tile.TileContext(nc) as tc:
        with (
            tc.tile_pool(name="sb", bufs=1) as sb,
            tc.tile_pool(name="ps", bufs=5, space="PSUM") as ps,
            tc.tile_pool(name="pt", bufs=2, space="PSUM") as pt,
        ):
            # ---- constants ----
            ident_f = sb.tile([128, 128], F32, tag="identf")
            make_identity(nc, ident_f)
            ident = sb.tile([128, 128], MD, tag="ident")
            nc.vector.tensor_copy(ident, ident_f)
            ones1_f = sb.tile([1, 128], F32, tag="ones1f")
            nc.vector.memset(ones1_f, 1.0)
            ones1 = sb.tile([1, 128], MD, tag="ones1")
            nc.vector.tensor_copy(ones1, ones1_f)

            wc_sb = sb.tile([128, NK], F32, tag="wc")
            wm_sb = sb.tile([128, NK], F32, tag="wm")
            wq_sb = sb.tile([128, NK], MD, tag="wq")
            nc.sync.dma_start(out=wc_sb, in_=w0[0:D].rearrange("(k p) -> p k", p=128))
            nc.sync.dma_start(out=wm_sb, in_=w0[2 * D:3 * D].rearrange("(k p) -> p k", p=128))
            load(wq_sb, w0[D:2 * D].rearrange("(k p) -> p k", p=128))

            br_sb = sb.tile([1, D], MD, tag="br")
            load(br_sb, br.rearrange("(a e) -> a e", a=1))

            W_sb = sb.tile([128, 4 * NK, D], MD, tag="W")
            wr_r = wr.rearrange("(t p) e -> p t e", p=128)
            for tq in range(4):
                load(W_sb[:, tq * NK:(tq + 1) * NK, :],
                     wr_r[:, tq * NK:(tq + 1) * NK, :])

            for b in range(BPC):
                # ---- load ----
                cN = sb.tile([128, NT, D], MD, tag="bigA")
                c_r = c2[b].rearrange("(t p) d -> p t d", p=128)
                for tq in range(4):
                    load(cN[:, tq * 4:(tq + 1) * 4, :],
                         c_r[:, tq * 4:(tq + 1) * 4, :])
                qN = sb.tile([128, NG, D], MD, tag="qN")
                load(qN, q2[b].rearrange("(g p) d -> p g d", p=128))

                # ---- transposes (PE transpose-mode through PSUM) ----
                qT = sb.tile([128, NK, Lq], MD, tag="qT")
                for kd in range(NK):
                    ptile = pt.tile([128, 512], MD, tag="tr")
                    for g in range(NG):
                        nc.tensor.transpose(
                            ptile[:, g * 128:(g + 1) * 128],
                            qN[:, g, kd * 128:(kd + 1) * 128], ident)
                    nc.any.tensor_copy(qT[:, kd, :], ptile)

                cT = sb.tile([128, NK, Lc], MD, tag="bigC")
                for kd in range(NK):
                    for ic in range(NC_):
                        ptile = pt.tile([128, 512], MD, tag="tr")
                        for t4 in range(4):
                            t = ic * 4 + t4
                            nc.tensor.transpose(
                                ptile[:, t4 * 128:(t4 + 1) * 128],
                                cN[:, t, kd * 128:(kd + 1) * 128], ident)
                        nc.any.tensor_copy(cT[:, kd, ic * 512:(ic + 1) * 512], ptile)

                # ---- v = q @ wq (1, Lq), from raw qT ----
                v_sb = sb.tile([1, Lq], MD, tag="v")
                pv = ps.tile([128, 512], F32, tag="mm")
                for kd in range(NK):
                    nc.tensor.matmul(pv[0:1, :], wq_sb[:, kd:kd + 1], qT[:, kd, :],
                                     start=(kd == 0), stop=(kd == NK - 1))
                nc.any.tensor_copy(v_sb, pv[0:1, :])

                # ---- q~T = wm * qT + wc (in place) ----
                for kd in range(NK):
                    nc.vector.tensor_scalar(
                        out=qT[:, kd, :], in0=qT[:, kd, :],
                        scalar1=wm_sb[:, kd:kd + 1], scalar2=wc_sb[:, kd:kd + 1],
                        op0=ALU.mult, op1=ALU.add)

                # ---- ST = q~T.T @ cT -> E1T = exp(ST); s1 row sums ----
                E1T = sb.tile([128, NG, Lc], MD, tag="bigD")
                s1p = sb.tile([128, NG, NC_], F32, tag="s1p")
                s1s = sb.tile([128, NG], F32, tag="s1s")
                invs1 = sb.tile([128, NG], F32, tag="invs1")
                for g in range(NG):
                    for ic in range(NC_):
                        pm = ps.tile([128, 512], F32, tag="mm")
                        for kd in range(NK):
                            nc.tensor.matmul(
                                pm, qT[:, kd, g * 128:(g + 1) * 128],
                                cT[:, kd, ic * 512:(ic + 1) * 512],
                                start=(kd == 0), stop=(kd == NK - 1))
                        nc.scalar.activation(
                            out=E1T[:, g, ic * 512:(ic + 1) * 512], in_=pm,
                            func=AF.Exp, accum_out=s1p[:, g, ic:ic + 1])
                    nc.vector.reduce_sum(out=s1s[:, g:g + 1], in_=s1p[:, g, :], axis=AX.X)
                    nc.vector.reciprocal(out=invs1[:, g:g + 1], in_=s1s[:, g:g + 1])

                # ---- S natural = cT.T @ q~T + ones x v -> S2 (normalized) ----
                S2 = sb.tile([128, NT, Lq], MD, tag="bigB")
                s2s = sb.tile([128, NT], F32, tag="s2s")
                invs2 = sb.tile([128, NT], F32, tag="invs2")
                for t in range(NT):
                    pm = ps.tile([128, 512], F32, tag="mm")
                    for kd in range(NK):
                        nc.tensor.matmul(
                            pm, cT[:, kd, t * 128:(t + 1) * 128], qT[:, kd, :],
                            start=(kd == 0), stop=False)
                    nc.tensor.matmul(pm, ones1, v_sb, start=False, stop=True)
                    nc.scalar.activation(
                        out=S2[:, t, :], in_=pm,
                        func=AF.Exp, accum_out=s2s[:, t:t + 1])
                    nc.vector.reciprocal(out=invs2[:, t:t + 1], in_=s2s[:, t:t + 1])
                    nc.vector.tensor_scalar_mul(S2[:, t, :], S2[:, t, :], invs2[:, t:t + 1])

                # ---- fold 1/s1 into qN rows ----
                for g in range(NG):
                    nc.vector.tensor_scalar_mul(qN[:, g, :], qN[:, g, :], invs1[:, g:g + 1])

                # ---- Y = (S2.T @ c) / s1 ----
                Y = sb.tile([128, NG, D], MD, tag="Y")
                for g in range(NG):
                    pm = ps.tile([128, 512], F32, tag="mm")
                    for t in range(NT):
                        nc.tensor.matmul(
                            pm, S2[:, t, g * 128:(g + 1) * 128], cN[:, t, :],
                            start=(t == 0), stop=(t == NT - 1))
                    nc.vector.tensor_scalar_mul(Y[:, g, :], pm, invs1[:, g:g + 1])

                # ---- AT = (q/s1).T @ E1T ----
                AT = sb.tile([128, NK, Lc], MD, tag="bigA")
                for kd in range(NK):
                    for ic in range(NC_):
                        pm = ps.tile([128, 512], F32, tag="mm")
                        for g in range(NG):
                            nc.tensor.matmul(
                                pm, qN[:, g, kd * 128:(kd + 1) * 128],
                                E1T[:, g, ic * 512:(ic + 1) * 512],
                                start=(g == 0), stop=(g == NG - 1))
                        nc.any.tensor_copy(AT[:, kd, ic * 512:(ic + 1) * 512], pm)

                # ---- BmT = Y.T @ E1T ----
                BmT = sb.tile([128, NK, Lc], MD, tag="bigB")
                for kd in range(NK):
                    for ic in range(NC_):
                        pm = ps.tile([128, 512], F32, tag="mm")
                        for g in range(NG):
                            nc.tensor.matmul(
                                pm, Y[:, g, kd * 128:(kd + 1) * 128],
                                E1T[:, g, ic * 512:(ic + 1) * 512],
                                start=(g == 0), stop=(g == NG - 1))
                        nc.any.tensor_copy(BmT[:, kd, ic * 512:(ic + 1) * 512], pm)

                # ---- cAT = cT * AT (new slot), cBT = cT * BmT (in place) ----
                cAT = sb.tile([128, NK, Lc], MD, tag="bigD")
                for kd in range(NK):
                    nc.vector.tensor_mul(cAT[:, kd, :], cT[:, kd, :], AT[:, kd, :])
                for kd in range(NK):
                    nc.vector.tensor_mul(BmT[:, kd, :], BmT[:, kd, :], cT[:, kd, :])

                # ---- out = c@W1 + A@W2 + cA@W3 + cB@W4 + br ----
                for t in range(NT):
                    pm = ps.tile([128, 512], F32, tag="mm")
                    first = True
                    for si, src in enumerate((cT, AT, cAT, BmT)):
                        for kd in range(NK):
                            nc.tensor.matmul(
                                pm, src[:, kd, t * 128:(t + 1) * 128],
                                W_sb[:, si * NK + kd, :], start=first, stop=False)
                            first = False
                    nc.tensor.matmul(pm, ones1, br_sb, start=False, stop=True)
                    ot = sb.tile([128, 512], F32, tag="outst", bufs=3)
                    nc.any.tensor_copy(ot, pm)
                    nc.sync.dma_start(out=out2[b, t * 128:(t + 1) * 128, :], in_=ot)

    nc.compile()
    return nc


class Runner:
    """Persistent SPMD runner: jit once, execute many times.

    Mirrors concourse.bass2jax.run_bass_via_pjrt's multi-core path but keeps
    the compiled executable so repeated calls don't recompile.
    """

    def __init__(self, nc):
        import jax
        from jax.experimental.shard_map import shard_map
        from jax.sharding import Mesh, PartitionSpec

        bass2jax.install_neuronx_cc_hook()
        self.nc = nc
        self.jax = jax

        partition_name = (
            nc.partition_id_tensor.name if nc.partition_id_tensor else None
        )
        in_names, out_names, out_avals, zero_shapes = [], [], [], []
        for alloc in nc.m.functions[0].allocations:
            if not isinstance(alloc, mybir.MemoryLocationSet):
                continue
            name = alloc.memorylocations[0].name
            if alloc.kind == "ExternalInput":
                if name != partition_name:
                    in_names.append(name)
            elif alloc.kind == "ExternalOutput":
                shape = tuple(alloc.tensor_shape)
                dtype = mybir.dt.np(alloc.dtype)
                out_names.append(name)
                out_avals.append(jax.core.ShapedArray(shape, dtype))
                zero_shapes.append((shape, dtype))
        self.in_names = list(in_names)
        self.out_names = out_names
        self.out_avals = out_avals
        self.zero_shapes = zero_shapes
        n_params = len(in_names)
        n_outs = len(out_names)

        all_in_names = list(in_names) + list(out_names)
        if partition_name is not None:
            all_in_names.append(partition_name)

        def _body(*args):
            operands = list(args)
            if partition_name is not None:
                operands.append(bass2jax.partition_id_tensor())
            outs = bass2jax._bass_exec_p.bind(
                *operands,
                out_avals=tuple(out_avals),
                in_names=tuple(all_in_names),
                out_names=tuple(out_names),
                lowering_input_output_aliases=(),
                sim_require_finite=True,
                sim_require_nnan=True,
                nc=nc,
            )
            return tuple(outs)

        devices = jax.devices()[:N_CORES]
        mesh = Mesh(np.asarray(devices), ("core",))
        in_specs = (PartitionSpec("core"),) * (n_params + n_outs)
        out_specs = (PartitionSpec("core"),) * n_outs
        self.fn = jax.jit(
            shard_map(_body, mesh=mesh, in_specs=in_specs,
                      out_specs=out_specs, check_rep=False),
            keep_unused=True,
        )

    def concat_inputs(self, in_maps):
        return [
            np.concatenate([np.asarray(m[name]) for m in in_maps], axis=0)
            for name in self.in_names
        ]

    def zeros(self):
        return [
            np.zeros((N_CORES * s[0], *s[1:]), d) for (s, d) in self.zero_shapes
        ]

    def run_device(self, concat_in, zeros):
        """Execute; returns list of global (N_CORES*dim0, ...) jax arrays."""
        out = self.fn(*concat_in, *zeros)
        self.jax.block_until_ready(out)
        return out

    def run(self, in_maps):
        outs = self.run_device(self.concat_inputs(in_maps), self.zeros())
        return [
            {
                name: np.asarray(outs[i]).reshape(
                    N_CORES, *self.out_avals[i].shape)[c]
                for i, name in enumerate(self.out_names)
            }
            for c in range(N_CORES)
        ]


_CACHED = {}


def _get_runner(**kw):
    key = tuple(sorted(kw.items()))
    if key not in _CACHED:
        _CACHED[key] = Runner(build_program(**kw))
    return _CACHED[key]


def make_in_maps(context, question, w0, wr, br):
    return [
        {
            "c2": context[c * BPC:(c + 1) * BPC],
            "q2": question[c * BPC:(c + 1) * BPC],
            "w0": w0,
            "wr": wr,
            "br": br,
        }
        for c in range(N_CORES)
    ]


def kernel(context, question, w0, wr, br):
    context = np.ascontiguousarray(np.asarray(context, dtype=np.float32))
    question = np.ascontiguousarray(np.asarray(question, dtype=np.float32))
    w0 = np.ascontiguousarray(np.asarray(w0, dtype=np.float32))
    wr = np.ascontiguousarray(np.asarray(wr, dtype=np.float32))
    br = np.ascontiguousarray(np.asarray(br, dtype=np.float32))

    runner = _get_runner()
    res = runner.run(make_in_maps(context, question, w0, wr, br))
    return np.concatenate([res[c]["out2"] for c in range(N_CORES)], axis=0)


# revision 9
# speedup vs baseline: 196.6129x; 196.6129x over previous
"""ContextQueryAttention Trainium2 kernel.

Reference computation (per batch b):
    S = (c@wc)[:,None] + (q@wq)[None,:] + (c*wm) @ q.T        # (Lc, Lq)
    S1 = softmax(S, axis=0)  (over context dim i)
    S2 = softmax(S, axis=1)  (over question dim j)
    A  = S1 @ q
    Bm = (S1 @ S2.T) @ c
    out = [c, A, c*A, c*Bm] @ wr + br

Algebraic restructuring used here:
  * Bm = S1 @ (S2.T @ c)   -- avoids the (Lc,Lc) intermediate entirely.
  * q~ = wm*q + wc (per-feature). Then q~ @ c.T = M^T + u[i] where
    u = c@wc; the v[j] = q@wq term is constant along the i-softmax and
    cancels, so S1^T = softmax_free(q~ @ c.T) directly.
  * c @ q~.T = M + u[i]; u[i] is constant along the j-softmax, and v[j]
    is added via a rank-1 matmul (ones x v) into the PSUM accumulation.
  * exp() without max subtraction (inputs are unit-scale gaussians; S
    stays |S| < ~10, far from fp32 overflow).
  * softmax normalizers are folded into downstream operands instead of
    rescaling the big exp(S^T) matrix: A = (q/s1) @ E1, Bm uses
    Y = (S2^T c)/s1.
  * The (Lc,4D)@(4D,D) output projection is done blockwise from the
    d-major (transposed) layouts produced naturally by the PE.
  * Matmuls run in float32r (e8m11, full PE rate) -- all PE operands are
    float32r-typed tiles so producers emit rounded values.

Sharding: pure data parallel over batch: 16 batches -> 8 cores x 2.
"""

import numpy as np

import concourse.bass as bass
import concourse.tile as tile
from concourse import bacc, mybir
from concourse import bass2jax
from concourse.masks import make_identity

N_CORES = 8
B, Lc, Lq, D = 16, 2048, 512, 512
BPC = B // N_CORES  # batches per core

F32 = mybir.dt.float32
F32R = mybir.dt.float32r

AF = mybir.ActivationFunctionType
ALU = mybir.AluOpType
AX = mybir.AxisListType

NT = Lc // 128   # 16 context row-blocks
NG = Lq // 128   # 4 question row-blocks
NK = D // 128    # 4 feature blocks
NC_ = Lc // 512  # 4 i-chunks of 512


def build_program(mm_f32r=True, repeat=1):
    MD = F32R if mm_f32r else F32  # dtype of every PE-consumed tile

    nc = bacc.Bacc(None, target_bir_lowering=False)

    c2 = nc.declare_dram_parameter("c2", [BPC, Lc, D], F32, isOutput=False)
    q2 = nc.declare_dram_parameter("q2", [BPC, Lq, D], F32, isOutput=False)
    w0 = nc.declare_dram_parameter("w0", [3 * D], F32, isOutput=False)
    wr = nc.declare_dram_parameter("wr", [4 * D, D], F32, isOutput=False)
    br = nc.declare_dram_parameter("br", [D], F32, isOutput=False)
    out2 = nc.declare_dram_parameter("out2", [BPC, Lc, D], F32, isOutput=True)

    # gpsimd (SWDGE) DMAs can cast f32 -> f32r on the fly; HWDGE cannot.
    def load(out, in_):
        if mm_f32r:
            nc.gpsimd.dma_start(out=out, in_=in_)
        else:
            nc.sync.dma_start(out=out, in_=in_)

    with tile.TileContext(nc) as tc:
        with (
            tc.tile_pool(name="sb", bufs=1) as sb,
            tc.tile_pool(name="ps", bufs=5, space="PSUM") as ps,
            tc.tile_pool(name="pt", bufs=2, space="PSUM") as pt,
        ):
            # ---- constants ----
            ident_f = sb.tile([128, 128], F32, tag="identf")
            make_identity(nc, ident_f)
            ident = sb.tile([128, 128], MD, tag="ident")
            nc.vector.tensor_copy(ident, ident_f)
            ones1_f = sb.tile([1, 128], F32, tag="ones1f")
            nc.vector.memset(ones1_f, 1.0)
            ones1 = sb.tile([1, 128], MD, tag="ones1")
            nc.vector.tensor_copy(ones1, ones1_f)

            wc_sb = sb.tile([128, NK], F32, tag="wc")
            wm_sb = sb.tile([128, NK], F32, tag="wm")
            wq_sb = sb.tile([128, NK], MD, tag="wq")
            nc.sync.dma_start(out=wc_sb, in_=w0[0:D].rearrange("(k p) -> p k", p=128))
            nc.sync.dma_start(out=wm_sb, in_=w0[2 * D:3 * D].rearrange("(k p) -> p k", p=128))
            load(wq_sb, w0[D:2 * D].rearrange("(k p) -> p k", p=128))

            br_sb = sb.tile([1, D], MD, tag="br")
            load(br_sb, br.rearrange("(a e) -> a e", a=1))

            W_sb = sb.tile([128, 4 * NK, D], MD, tag="W")
            wr_r = wr.rearrange("(t p) e -> p t e", p=128)
            for tq in range(4):
                load(W_sb[:, tq * NK:(tq + 1) * NK, :],
                     wr_r[:, tq * NK:(tq + 1) * NK, :])

            def one_batch(b):
                # ---- load ----
                cN = sb.tile([128, NT, D], MD, tag="bigA")
                c_r = c2[b].rearrange("(t p) d -> p t d", p=128)
                for tq in range(4):
                    load(cN[:, tq * 4:(tq + 1) * 4, :],
                         c_r[:, tq * 4:(tq + 1) * 4, :])
                qN = sb.tile([128, NG, D], MD, tag="qN")
                load(qN, q2[b].rearrange("(g p) d -> p g d", p=128))

                # ---- transposes (PE transpose-mode through PSUM) ----
                qT = sb.tile([128, NK, Lq], MD, tag="qT")
                for kd in range(NK):
                    ptile = pt.tile([128, 512], MD, tag="tr")
                    for g in range(NG):
                        nc.tensor.transpose(
                            ptile[:, g * 128:(g + 1) * 128],
                            qN[:, g, kd * 128:(kd + 1) * 128], ident)
                    nc.any.tensor_copy(qT[:, kd, :], ptile)

                cT = sb.tile([128, NK, Lc], MD, tag="bigC")
                for kd in range(NK):
                    for ic in range(NC_):
                        ptile = pt.tile([128, 512], MD, tag="tr")
                        for t4 in range(4):
                            t = ic * 4 + t4
                            nc.tensor.transpose(
                                ptile[:, t4 * 128:(t4 + 1) * 128],
                                cN[:, t, kd * 128:(kd + 1) * 128], ident)
                        nc.any.tensor_copy(cT[:, kd, ic * 512:(ic + 1) * 512], ptile)

                # ---- v = q @ wq (1, Lq), from raw qT ----
                v_sb = sb.tile([1, Lq], MD, tag="v")
                pv = ps.tile([128, 512], F32, tag="mm")
                for kd in range(NK):
                    nc.tensor.matmul(pv[0:1, :], wq_sb[:, kd:kd + 1], qT[:, kd, :],
                                     start=(kd == 0), stop=(kd == NK - 1))
                nc.any.tensor_copy(v_sb, pv[0:1, :])

                # ---- q~T = wm * qT + wc (in place) ----
                for kd in range(NK):
                    nc.vector.tensor_scalar(
                        out=qT[:, kd, :], in0=qT[:, kd, :],
                        scalar1=wm_sb[:, kd:kd + 1], scalar2=wc_sb[:, kd:kd + 1],
                        op0=ALU.mult, op1=ALU.add)

                # ---- ST = q~T.T @ cT -> E1T = exp(ST); s1 row sums ----
                E1T = sb.tile([128, NG, Lc], MD, tag="bigD")
                s1p = sb.tile([128, NG, NC_], F32, tag="s1p")
                s1s = sb.tile([128, NG], F32, tag="s1s")
                invs1 = sb.tile([128, NG], F32, tag="invs1")
                for g in range(NG):
                    for ic in range(NC_):
                        pm = ps.tile([128, 512], F32, tag="mm")
                        for kd in range(NK):
                            nc.tensor.matmul(
                                pm, qT[:, kd, g * 128:(g + 1) * 128],
                                cT[:, kd, ic * 512:(ic + 1) * 512],
                                start=(kd == 0), stop=(kd == NK - 1))
                        nc.scalar.activation(
                            out=E1T[:, g, ic * 512:(ic + 1) * 512], in_=pm,
                            func=AF.Exp, accum_out=s1p[:, g, ic:ic + 1])
                    nc.vector.reduce_sum(out=s1s[:, g:g + 1], in_=s1p[:, g, :], axis=AX.X)
                    nc.vector.reciprocal(out=invs1[:, g:g + 1], in_=s1s[:, g:g + 1])

                # ---- S natural = cT.T @ q~T + ones x v -> S2 (normalized) ----
                S2 = sb.tile([128, NT, Lq], MD, tag="bigB")
                s2s = sb.tile([128, NT], F32, tag="s2s")
                invs2 = sb.tile([128, NT], F32, tag="invs2")
                for t in range(NT):
                    pm = ps.tile([128, 512], F32, tag="mm")
                    for kd in range(NK):
                        nc.tensor.matmul(
                            pm, cT[:, kd, t * 128:(t + 1) * 128], qT[:, kd, :],
                            start=(kd == 0), stop=False)
                    nc.tensor.matmul(pm, ones1, v_sb, start=False, stop=True)
                    nc.scalar.activation(
                        out=S2[:, t, :], in_=pm,
                        func=AF.Exp, accum_out=s2s[:, t:t + 1])
                    nc.vector.reciprocal(out=invs2[:, t:t + 1], in_=s2s[:, t:t + 1])
                    nc.vector.tensor_scalar_mul(S2[:, t, :], S2[:, t, :], invs2[:, t:t + 1])

                # ---- fold 1/s1 into qN rows ----
                for g in range(NG):
                    nc.vector.tensor_scalar_mul(qN[:, g, :], qN[:, g, :], invs1[:, g:g + 1])

                # ---- Y = (S2.T @ c) / s1 ----
                Y = sb.tile([128, NG, D], MD, tag="Y")
                for g in range(NG):
                    pm = ps.tile([128, 512], F32, tag="mm")
                    for t in range(NT):
                        nc.tensor.matmul(
                            pm, S2[:, t, g * 128:(g + 1) * 128], cN[:, t, :],
                            start=(t == 0), stop=(t == NT - 1))
                    nc.vector.tensor_scalar_mul(Y[:, g, :], pm, invs1[:, g:g + 1])

                # ---- AT = (q/s1).T @ E1T ----
                AT = sb.tile([128, NK, Lc], MD, tag="bigA")
                for kd in range(NK):
                    for ic in range(NC_):
                        pm = ps.tile([128, 512], F32, tag="mm")
                        for g in range(NG):
                            nc.tensor.matmul(
                                pm, qN[:, g, kd * 128:(kd + 1) * 128],
                                E1T[:, g, ic * 512:(ic + 1) * 512],
                                start=(g == 0), stop=(g == NG - 1))
                        nc.any.tensor_copy(AT[:, kd, ic * 512:(ic + 1) * 512], pm)

                # ---- BmT = Y.T @ E1T ----
                BmT = sb.tile([128, NK, Lc], MD, tag="bigB")
                for kd in range(NK):
                    for ic in range(NC_):
                        pm = ps.tile([128, 512], F32, tag="mm")
                        for g in range(NG):
                            nc.tensor.matmul(
                                pm, Y[:, g, kd * 128:(kd + 1) * 128],
                                E1T[:, g, ic * 512:(ic + 1) * 512],
                                start=(g == 0), stop=(g == NG - 1))
                        nc.any.tensor_copy(BmT[:, kd, ic * 512:(ic + 1) * 512], pm)

                # ---- cAT = cT * AT (new slot), cBT = cT * BmT (in place) ----
                cAT = sb.tile([128, NK, Lc], MD, tag="bigD")
                for kd in range(NK):
                    nc.vector.tensor_mul(cAT[:, kd, :], cT[:, kd, :], AT[:, kd, :])
                for kd in range(NK):
                    nc.vector.tensor_mul(BmT[:, kd, :], BmT[:, kd, :], cT[:, kd, :])

                # ---- out = c@W1 + A@W2 + cA@W3 + cB@W4 + br ----
                for t in range(NT):
                    pm = ps.tile([128, 512], F32, tag="mm")
                    first = True
                    for si, src in enumerate((cT, AT, cAT, BmT)):
                        for kd in range(NK):
                            nc.tensor.matmul(
                                pm, src[:, kd, t * 128:(t + 1) * 128],
                                W_sb[:, si * NK + kd, :], start=first, stop=False)
                            first = False
                    nc.tensor.matmul(pm, ones1, br_sb, start=False, stop=True)
                    ot = sb.tile([128, 512], F32, tag="outst", bufs=3)
                    nc.any.tensor_copy(ot, pm)
                    nc.sync.dma_start(out=out2[b, t * 128:(t + 1) * 128, :], in_=ot)

            if repeat > 1:
                # timing harness only: repeat the whole workload on-device so
                # per-call dispatch overhead can be subtracted out
                hints = (mybir.EngineType.PE, mybir.EngineType.DVE,
                         mybir.EngineType.Activation, mybir.EngineType.SP,
                         mybir.EngineType.Pool)
                with tc.For_i(0, repeat, 1, hint_engines=hints):
                    for b in range(BPC):
                        one_batch(b)
            else:
                for b in range(BPC):
                    one_batch(b)

    nc.compile()
    return nc


class Runner:
    """Persistent SPMD runner: jit once, execute many times.

    Mirrors concourse.bass2jax.run_bass_via_pjrt's multi-core path but keeps
    the compiled executable so repeated calls don't recompile.
    """

    def __init__(self, nc):
        import jax
        from jax.experimental.shard_map import shard_map
        from jax.sharding import Mesh, PartitionSpec

        bass2jax.install_neuronx_cc_hook()
        self.nc = nc
        self.jax = jax

        partition_name = (
            nc.partition_id_tensor.name if nc.partition_id_tensor else None
        )
        in_names, out_names, out_avals, zero_shapes = [], [], [], []
        for alloc in nc.m.functions[0].allocations:
            if not isinstance(alloc, mybir.MemoryLocationSet):
                continue
            name = alloc.memorylocations[0].name
            if alloc.kind == "ExternalInput":
                if name != partition_name:
                    in_names.append(name)
            elif alloc.kind == "ExternalOutput":
                shape = tuple(alloc.tensor_shape)
                dtype = mybir.dt.np(alloc.dtype)
                out_names.append(name)
                out_avals.append(jax.core.ShapedArray(shape, dtype))
                zero_shapes.append((shape, dtype))
        self.in_names = list(in_names)
        self.out_names = out_names
        self.out_avals = out_avals
        self.zero_shapes = zero_shapes
        n_params = len(in_names)
        n_outs = len(out_names)

        all_in_names = list(in_names) + list(out_names)
        if partition_name is not None:
            all_in_names.append(partition_name)

        def _body(*args):
            operands = list(args)
            if partition_name is not None:
                operands.append(bass2jax.partition_id_tensor())
            outs = bass2jax._bass_exec_p.bind(
                *operands,
                out_avals=tuple(out_avals),
                in_names=tuple(all_in_names),
                out_names=tuple(out_names),
                lowering_input_output_aliases=(),
                sim_require_finite=True,
                sim_require_nnan=True,
                nc=nc,
            )
            return tuple(outs)

        devices = jax.devices()[:N_CORES]
        mesh = Mesh(np.asarray(devices), ("core",))
        in_specs = (PartitionSpec("core"),) * (n_params + n_outs)
        out_specs = (PartitionSpec("core"),) * n_outs
        self.fn = jax.jit(
            shard_map(_body, mesh=mesh, in_specs=in_specs,
                      out_specs=out_specs, check_rep=False),
            keep_unused=True,
        )

    def concat_inputs(self, in_maps):
        return [
            np.concatenate([np.asarray(m[name]) for m in in_maps], axis=0)
            for name in self.in_names
        ]

    def zeros(self):
        return [
            np.zeros((N_CORES * s[0], *s[1:]), d) for (s, d) in self.zero_shapes
        ]

    def run_device(self, concat_in, zeros):
        """Execute; returns list of global (N_CORES*dim0, ...) jax arrays."""
        out = self.fn(*concat_in, *zeros)
        self.jax.block_until_ready(out)
        return out

    def run(self, in_maps):
        outs = self.run_device(self.concat_inputs(in_maps), self.zeros())
        return [
            {
                name: np.asarray(outs[i]).reshape(
                    N_CORES, *self.out_avals[i].shape)[c]
                for i, name in enumerate(self.out_names)
            }
            for c in range(N_CORES)
        ]


_CACHED = {}


def _get_runner(**kw):
    key = tuple(sorted(kw.items()))
    if key not in _CACHED:
        _CACHED[key] = Runner(build_program(**kw))
    return _CACHED[key]


def make_in_maps(context, question, w0, wr, br):
    return [
        {
            "c2": context[c * BPC:(c + 1) * BPC],
            "q2": question[c * BPC:(c + 1) * BPC],
            "w0": w0,
            "wr": wr,
            "br": br,
        }
        for c in range(N_CORES)
    ]


def kernel(context, question, w0, wr, br):
    context = np.ascontiguousarray(np.asarray(context, dtype=np.float32))
    question = np.ascontiguousarray(np.asarray(question, dtype=np.float32))
    w0 = np.ascontiguousarray(np.asarray(w0, dtype=np.float32))
    wr = np.ascontiguousarray(np.asarray(wr, dtype=np.float32))
    br = np.ascontiguousarray(np.asarray(br, dtype=np.float32))

    runner = _get_runner()
    res = runner.run(make_in_maps(context, question, w0, wr, br))
    return np.concatenate([res[c]["out2"] for c in range(N_CORES)], axis=0)
